# revision 29
# baseline (speedup 1.0000x reference)
"""Trainium2 Bass kernel for nn_DetectionTargetLayer (Mask R-CNN detection
target generation). Self-contained: builds an SPMD Bass/Tile program, shards
the batch over 8 NeuronCores (2 cores per image, positive-ROI mask work split
by slot parity via a permutation constant), runs via run_bass_kernel_spmd,
and reassembles full outputs on the host.

Numerics: every value-bearing path is exact — indicator/0-1 matmuls, PE
transpose mode (permutation datapath), indirect-DMA row gathers, and fp32 DVE
ops replicating the reference's operation order. See model.py (dev repo) for
the numpy twin validated bitwise against the jax reference.
"""
import sys

for p in ("/opt/trn_rl_repo", "/root/.axon_site/_ro/trn_rl_repo"):
    if p not in sys.path:
        sys.path.insert(0, p)

import numpy as np
import bass_rust
import concourse.bass as bass
import concourse.bacc as bacc
import concourse.mybir as mybir
import concourse.tile as tile
from concourse.bass import ds
from concourse.bass_utils import run_bass_kernel_spmd

F32 = mybir.dt.float32
BF16 = mybir.dt.bfloat16
I32 = mybir.dt.int32
U8 = mybir.dt.uint8
ALU = mybir.AluOpType
ACT = mybir.ActivationFunctionType
AX = mybir.AxisListType

NP = 2000
PT = 125          # proposals per tile
NT = 16           # proposal tiles
NG = 64           # max GT
NPOS = 66
NNEG = 134        # 128 + 6
WIN = 192         # mask x-window (max span 164.5px + margin)
MH = 28
EPS = 1e-8
BIG = 65536.0


# ---------------------------------------------------------------- tail-drain
# walrus in this env caps non-EventSemaphore instructions at one sync wait;
# stock TileContext attaches every outstanding clock wait to one SP drain.
def _patched_drain_and_barrier(self, tick_clock, wait_clock):
    drain_inst = self.nc.sync.drain()
    wait_clock.add_sem_waits(
        drain_inst.ins, tile.ScopedClock({None: tick_clock.global_clock})
    )
    raw = drain_inst.ins
    si = raw.sync_info
    waits = list(si.on_wait) if si is not None else []
    if len(waits) > 1:
        raw.sync_info = bass_rust.SyncInfo(
            on_wait=[waits[0]], on_update=list(si.on_update)
        )
        for extra in waits[1:]:
            d2 = self.nc.sync.drain()
            d2.ins.sync_info = bass_rust.SyncInfo(on_wait=[extra], on_update=[])
    self.nc.all_engine_barrier()
    assert self.sems is not None
    popped = self.nc._tile_sem_poison_stack.pop()
    assert popped is self._sem_poison
    self.nc.clear_and_free_semaphores(list(self.sems.allocated().values()))
    self.nc.all_engine_barrier()


tile.TileContext._drain_and_barrier = _patched_drain_and_barrier


# ---------------------------------------------------------------- constants
def host_constants(parity: int) -> dict:
    cf = np.zeros((128, 8), np.float32)
    cf[:, 0] = np.arange(128)
    cf[:, 1] = np.arange(128) + 128
    cf[:, 2] = np.arange(128) + 256
    cf[:, 3] = np.arange(128) + 384
    cf[:, 4] = 1.0

    trio = np.zeros((128, 128), np.float32)
    k = np.arange(128)[:, None]
    c = np.arange(128)[None, :]
    trio[(k < c) & (k < PT) & (c < PT)] = 1.0
    trio[(k < PT) & (c == PT)] = 1.0

    onesf = np.ones((1, 128), np.float32)
    iotarow = np.arange(144, dtype=np.float32).reshape(1, 144)

    pidx2 = np.zeros((128, 32), np.float32)
    for t in range(NT):
        pidx2[:, 2 * t] = np.arange(128)
        pidx2[:, 2 * t + 1] = t

    negtab = np.zeros((1, 128), np.float32)
    for pc in range(67):
        negtab[0, pc] = float(int(np.float32(pc) / np.float32(0.33)) - pc)

    perm = np.zeros((NPOS, NPOS), np.float32)
    for j in range(NPOS):
        pj = 2 * j + parity if j < 33 else 2 * (j - 33) + (1 - parity)
        perm[pj, j] = 1.0

    idf = np.eye(128, dtype=np.float32)
    idh = np.eye(128, dtype=np.float32)  # cast to bf16 at upload

    iy = (np.arange(MH, dtype=np.float32) / np.float32(27.0)).astype(np.float32)
    iyrep = np.broadcast_to(iy, (NPOS, MH)).copy()

    return dict(cf=cf, trio=trio, onesf=onesf, iotarow=iotarow, pidx2=pidx2,
                negtab=negtab, perm=perm, idf=idf,
                idh=idh.astype(np.float32), iyrep=iyrep)


# ---------------------------------------------------------------- program
def build_nc() -> bass.Bass:
    nc = bacc.Bacc()
    prop_d = nc.declare_dram_parameter("prop", [NP, 4], F32, isOutput=False)
    gt_d = nc.declare_dram_parameter("gt", [NG, 4], F32, isOutput=False)
    cls_d = nc.declare_dram_parameter("clsf", [NG, 1], F32, isOutput=False)
    masks_d = nc.declare_dram_parameter("masks", [512, 512, NG], U8, isOutput=False)

    cf_d = nc.declare_dram_parameter("cf", [128, 8], F32, isOutput=False)
    trio_d = nc.declare_dram_parameter("trio", [128, 128], F32, isOutput=False)
    onesf_d = nc.declare_dram_parameter("onesf", [1, 128], F32, isOutput=False)
    iotarow_d = nc.declare_dram_parameter("iotarow", [1, 144], F32, isOutput=False)
    pidx2_d = nc.declare_dram_parameter("pidx2", [128, 32], F32, isOutput=False)
    negtab_d = nc.declare_dram_parameter("negtab", [1, 128], F32, isOutput=False)
    perm_d = nc.declare_dram_parameter("perm", [NPOS, NPOS], F32, isOutput=False)
    idf_d = nc.declare_dram_parameter("idf", [128, 128], F32, isOutput=False)
    idh_d = nc.declare_dram_parameter("idh", [128, 128], BF16, isOutput=False)
    iyrep_d = nc.declare_dram_parameter("iyrep", [NPOS, MH], F32, isOutput=False)

    orois_d = nc.declare_dram_parameter("orois", [200, 4], F32, isOutput=True)
    otcls_d = nc.declare_dram_parameter("otcls", [200, 1], F32, isOutput=True)
    otdel_d = nc.declare_dram_parameter("otdel", [200, 4], F32, isOutput=True)
    omask_d = nc.declare_dram_parameter("omask", [33, MH, MH], F32, isOutput=True)

    with tile.TileContext(nc) as tc:
        build_program(nc, tc, locals())
    return nc


def build_program(nc, tc, d):
    from contextlib import ExitStack

    V = nc.vector          # DVE
    S = nc.scalar          # ACT
    PE = nc.tensor

    def vload(ap, lo, hi):
        """value_load minus the runtime-assert emission (broken in this env)."""
        reg = V.alloc_register(f"vl_{nc.next_id()}")
        V.reg_load(reg, ap)
        return V.snap(reg, donate=True, min_val=lo, max_val=hi)

    stack = ExitStack()
    cpool = stack.enter_context(tc.tile_pool(name="const", bufs=1))
    persist = stack.enter_context(tc.tile_pool(name="persist", bufs=1))

    def load_const(name, shape, dtype=F32):
        t = cpool.tile(shape, dtype, name=name, tag=name)
        nc.sync.dma_start(t[:], d[name + "_d"][:])
        return t

    cf = load_const("cf", [128, 8])
    trio = load_const("trio", [128, 128])
    onesf = load_const("onesf", [1, 128])
    iotarow = load_const("iotarow", [1, 144])
    pidx2 = load_const("pidx2", [128, 32])
    negtab = load_const("negtab", [1, 128])
    perm = load_const("perm", [NPOS, NPOS])
    idf = load_const("idf", [128, 128])
    idh = load_const("idh", [128, 128], BF16)
    iyrep = load_const("iyrep", [NPOS, MH])

    prop_sb = persist.tile([PT, NT, 4], F32)
    nc.sync.dma_start(prop_sb[:], d["prop_d"][:].rearrange("(t p) c -> p t c", p=PT))
    gt_sb = persist.tile([NG, 4], F32)
    nc.sync.dma_start(gt_sb[:], d["gt_d"][:])
    cls_sb = persist.tile([NG, 1], F32)
    nc.sync.dma_start(cls_sb[:], d["cls_d"][:])

    # big mask tiles, one per 128-row y block
    mask_sb = []
    for b in range(4):
        mt = persist.tile([128, 512, NG], U8, tag=f"masks{b}")
        nc.sync.dma_start(mt[:], d["masks_d"][b * 128:(b + 1) * 128, :, :])
        mask_sb.append(mt)

    # ---------------- gt-side rows + broadcast [125, 512]
    with tc.tile_pool(name="sel", bufs=1) as sp, \
         tc.tile_pool(name="selp", bufs=1, space="PSUM") as pp:
        boxmax = sp.tile([NG, 1], F32)
        V.tensor_reduce(boxmax[:], gt_sb[:], AX.X, ALU.max)
        boxv = sp.tile([NG, 1], F32)
        V.tensor_scalar(out=boxv[:], in0=boxmax[:], scalar1=0.0, scalar2=None, op0=ALU.is_gt)
        flags8 = sp.tile([NG, 8], F32)
        V.tensor_copy(out=flags8[:, 0:4], in_=gt_sb[:])
        tmp = sp.tile([NG, 1], F32, tag="gtmp")
        V.tensor_scalar(out=tmp[:], in0=cls_sb[:], scalar1=0.0, scalar2=None, op0=ALU.is_gt)
        V.tensor_tensor(out=flags8[:, 4:5], in0=tmp[:], in1=boxv[:], op=ALU.mult)
        V.tensor_scalar(out=tmp[:], in0=cls_sb[:], scalar1=0.0, scalar2=None, op0=ALU.is_lt)
        V.tensor_tensor(out=flags8[:, 5:6], in0=tmp[:], in1=boxv[:], op=ALU.mult)
        t1 = sp.tile([NG, 1], F32, tag="gtmp2")
        V.tensor_tensor(out=t1[:], in0=gt_sb[:, 2:3], in1=gt_sb[:, 0:1], op=ALU.subtract)
        V.tensor_tensor(out=tmp[:], in0=gt_sb[:, 3:4], in1=gt_sb[:, 1:2], op=ALU.subtract)
        V.tensor_tensor(out=flags8[:, 6:7], in0=t1[:], in1=tmp[:], op=ALU.mult)
        V.memset(flags8[:, 7:8], 0.0)

        p8 = pp.tile([8, NG], F32, space="PSUM")
        PE.transpose(out=p8[:], in_=flags8[:], identity=idf[0:NG, 0:NG])
        gtrows = sp.tile([8, NG], F32)
        V.tensor_copy(out=gtrows[:], in_=p8[:])

        gtrow1 = sp.tile([1, 512], F32)
        nc.sync.dma_start(gtrow1[:], gtrows[:])
        pbc = pp.tile([PT, 512], F32, space="PSUM", tag="pbc")
        PE.matmul(out=pbc[:], lhsT=onesf[0:1, 0:PT], rhs=gtrow1[:], start=True, stop=True)
        bc = persist.tile([PT, 512], F32)
        V.tensor_copy(out=bc[:], in_=pbc[:])

    BY1, BX1, BY2, BX2 = bc[:, 0:64], bc[:, 64:128], bc[:, 128:192], bc[:, 192:256]
    BGTV, BCRWD, BA2 = bc[:, 256:320], bc[:, 320:384], bc[:, 384:448]

    gtvm1 = persist.tile([PT, 64], F32)
    V.tensor_scalar(out=gtvm1[:], in0=BGTV, scalar1=1.0, scalar2=None, op0=ALU.subtract)
    crwdm1 = persist.tile([PT, 64], F32)
    V.tensor_scalar(out=crwdm1[:], in0=BCRWD, scalar1=1.0, scalar2=None, op0=ALU.subtract)

    # iota broadcast [125, 144]
    with tc.tile_pool(name="kbp", bufs=1, space="PSUM") as kp:
        pkb = kp.tile([PT, 144], F32, space="PSUM")
        PE.matmul(out=pkb[:], lhsT=onesf[0:1, 0:PT], rhs=iotarow[:], start=True, stop=True)
        kb = persist.tile([PT, 144], F32)
        V.tensor_copy(out=kb[:], in_=pkb[:])

    # ---------------- per-tile IoU + flags
    flagsPN = persist.tile([PT, 2 * NT], F32)

    def iou_rows(pool, n, tag, py1, px1, py2, px2):
        """returns (iou_tile, inter_tile_reuse) for rows [n, 64]"""
        ih = pool.tile([n, 64], F32, tag=tag + "ih")
        tt = pool.tile([n, 64], F32, tag=tag + "tt")
        V.tensor_scalar(out=ih[:], in0=BY2[0:n, :], scalar1=py2, scalar2=None, op0=ALU.min)
        V.tensor_scalar(out=tt[:], in0=BY1[0:n, :], scalar1=py1, scalar2=None, op0=ALU.max)
        V.tensor_tensor(out=ih[:], in0=ih[:], in1=tt[:], op=ALU.subtract)
        V.tensor_scalar(out=ih[:], in0=ih[:], scalar1=0.0, scalar2=None, op0=ALU.max)
        iw = pool.tile([n, 64], F32, tag=tag + "iw")
        V.tensor_scalar(out=iw[:], in0=BX2[0:n, :], scalar1=px2, scalar2=None, op0=ALU.min)
        V.tensor_scalar(out=tt[:], in0=BX1[0:n, :], scalar1=px1, scalar2=None, op0=ALU.max)
        V.tensor_tensor(out=iw[:], in0=iw[:], in1=tt[:], op=ALU.subtract)
        V.tensor_scalar(out=iw[:], in0=iw[:], scalar1=0.0, scalar2=None, op0=ALU.max)
        V.tensor_tensor(out=ih[:], in0=ih[:], in1=iw[:], op=ALU.mult)  # inter
        a1 = pool.tile([n, 1], F32, tag=tag + "a1")
        b1 = pool.tile([n, 1], F32, tag=tag + "b1")
        V.tensor_tensor(out=a1[:], in0=py2, in1=py1, op=ALU.subtract)
        V.tensor_tensor(out=b1[:], in0=px2, in1=px1, op=ALU.subtract)
        V.tensor_tensor(out=a1[:], in0=a1[:], in1=b1[:], op=ALU.mult)
        # denom = a2 + a1 - inter + eps
        V.tensor_scalar(out=iw[:], in0=BA2[0:n, :], scalar1=a1[:], scalar2=None, op0=ALU.add)
        V.tensor_tensor(out=iw[:], in0=iw[:], in1=ih[:], op=ALU.subtract)
        V.tensor_scalar(out=iw[:], in0=iw[:], scalar1=float(EPS), scalar2=None, op0=ALU.add)
        V.reciprocal(out=iw[:], in_=iw[:])
        V.tensor_tensor(out=ih[:], in0=ih[:], in1=iw[:], op=ALU.mult)  # iou
        return ih

    with tc.tile_pool(name="iou", bufs=2) as ip:
        for t in range(NT):
            py1 = prop_sb[:, t, 0:1]
            px1 = prop_sb[:, t, 1:2]
            py2 = prop_sb[:, t, 2:3]
            px2 = prop_sb[:, t, 3:4]
            iou = iou_rows(ip, PT, "i", py1, px1, py2, px2)
            ov = ip.tile([PT, 64], F32, tag="ov")
            V.tensor_tensor(out=ov[:], in0=iou[:], in1=BGTV, op=ALU.mult)
            V.tensor_tensor(out=ov[:], in0=ov[:], in1=gtvm1[:], op=ALU.add)
            rmax = ip.tile([PT, 1], F32, tag="rmax")
            V.tensor_reduce(rmax[:], ov[:], AX.X, ALU.max)
            V.tensor_tensor(out=ov[:], in0=iou[:], in1=BCRWD, op=ALU.mult)
            V.tensor_tensor(out=ov[:], in0=ov[:], in1=crwdm1[:], op=ALU.add)
            cmax = ip.tile([PT, 1], F32, tag="cmax")
            V.tensor_reduce(cmax[:], ov[:], AX.X, ALU.max)
            propv = ip.tile([PT, 1], F32, tag="propv")
            V.tensor_reduce(propv[:], prop_sb[:, t, :], AX.X, ALU.max)
            V.tensor_scalar(out=propv[:], in0=propv[:], scalar1=0.0, scalar2=None, op0=ALU.is_gt)
            pf = ip.tile([PT, 1], F32, tag="pf")
            V.tensor_scalar(out=pf[:], in0=rmax[:], scalar1=0.5, scalar2=None, op0=ALU.is_ge)
            V.tensor_tensor(out=flagsPN[:, 2 * t:2 * t + 1], in0=pf[:], in1=propv[:], op=ALU.mult)
            V.tensor_scalar(out=rmax[:], in0=rmax[:], scalar1=0.5, scalar2=None, op0=ALU.is_lt)
            V.tensor_scalar(out=cmax[:], in0=cmax[:], scalar1=0.001, scalar2=None, op0=ALU.is_lt)
            V.tensor_tensor(out=rmax[:], in0=rmax[:], in1=cmax[:], op=ALU.mult)
            V.tensor_tensor(out=flagsPN[:, 2 * t + 1:2 * t + 2], in0=rmax[:], in1=propv[:], op=ALU.mult)

    # ---------------- ranks
    rk = persist.tile([125, 2 * NT], F32)
    tots = persist.tile([1, 2 * NT], F32)   # interleaved P/N totals
    with tc.tile_pool(name="rkp", bufs=2, space="PSUM") as rp:
        ptot = rp.tile([1, 2 * NT], F32, space="PSUM", tag="ptot", bufs=1)
        for t in range(NT):
            prk = rp.tile([125, 2], F32, space="PSUM", tag="prk")
            PE.matmul(out=prk[:], lhsT=trio[0:PT, 0:125], rhs=flagsPN[:, 2 * t:2 * t + 2],
                      start=True, stop=True)
            V.tensor_copy(out=rk[:, 2 * t:2 * t + 2], in_=prk[:])
            PE.matmul(out=ptot[:, 2 * t:2 * t + 2], lhsT=cf[0:PT, 4:5],
                      rhs=flagsPN[:, 2 * t:2 * t + 2], start=True, stop=True)
        V.tensor_copy(out=tots[:], in_=ptot[:])
    totP = tots[:].rearrange("p (t c) -> p c t", c=2)[:, 0, :]
    totN = tots[:].rearrange("p (t c) -> p c t", c=2)[:, 1, :]
    cums = persist.tile([1, 2 * NT], F32)   # [incP(16) | incN(16)]
    V.tensor_tensor_scan(out=cums[:, 0:NT], data0=totP, data1=totP,
                         initial=0.0, op0=ALU.add, op1=ALU.bypass)
    V.tensor_tensor_scan(out=cums[:, NT:2 * NT], data0=totN, data1=totN,
                         initial=0.0, op0=ALU.add, op1=ALU.bypass)
    excs = persist.tile([1, 2 * NT], F32)
    V.tensor_tensor(out=excs[:, 0:NT], in0=cums[:, 0:NT], in1=totP, op=ALU.subtract)
    V.tensor_tensor(out=excs[:, NT:2 * NT], in0=cums[:, NT:2 * NT], in1=totN, op=ALU.subtract)

    pcnt = persist.tile([1, 1], F32)
    V.tensor_scalar(out=pcnt[:], in0=cums[:, NT - 1:NT], scalar1=66.0, scalar2=None, op0=ALU.min)
    negavail = persist.tile([1, 1], F32)
    V.tensor_scalar(out=negavail[:], in0=cums[:, 2 * NT - 1:2 * NT], scalar1=134.0,
                    scalar2=None, op0=ALU.min)
    pcnt_i = persist.tile([1, 1], I32)
    V.tensor_copy(out=pcnt_i[:], in_=pcnt[:])
    negcnt = persist.tile([1, 1], F32)
    pcv = vload(pcnt_i[0:1, 0:1], 0, 66)
    V.tensor_copy(out=negcnt[:], in_=negtab[0:1, ds(pcv, 1)])
    V.tensor_scalar(out=negcnt[:], in0=negcnt[:], scalar1=0.0, scalar2=None, op0=ALU.max)
    V.tensor_tensor(out=negcnt[:], in0=negcnt[:], in1=negavail[:], op=ALU.min)

    # broadcasts of offsets / counts
    offs = persist.tile([PT, 2 * NT], F32)
    posv = persist.tile([NPOS, 1], F32)
    negvA = persist.tile([128, 1], F32)
    cnt2 = persist.tile([1, 2], F32)
    V.tensor_copy(out=cnt2[:, 0:1], in_=pcnt[:])
    V.tensor_copy(out=cnt2[:, 1:2], in_=negcnt[:])
    with tc.tile_pool(name="obp", bufs=1, space="PSUM") as op_:
        poffs = op_.tile([PT, 2 * NT], F32, space="PSUM", tag="poffs")
        PE.matmul(out=poffs[:], lhsT=onesf[0:1, 0:PT], rhs=excs[:], start=True, stop=True)
        V.tensor_copy(out=offs[:], in_=poffs[:])
        pcb = op_.tile([128, 2], F32, space="PSUM", tag="pcb")
        PE.matmul(out=pcb[:], lhsT=onesf[0:1, :], rhs=cnt2[:], start=True, stop=True)
        V.tensor_tensor(out=posv[:], in0=cf[0:NPOS, 0:1], in1=pcb[0:NPOS, 0:1], op=ALU.is_lt)
        V.tensor_tensor(out=negvA[:], in0=cf[:, 0:1], in1=pcb[:, 1:2], op=ALU.is_lt)
    negvB6 = persist.tile([6, 1], F32)
    # iota 128..133 vs negcnt broadcast (reuse posv psum? recompute cheaply)
    with tc.tile_pool(name="ob2", bufs=1, space="PSUM") as op2:
        pcb6 = op2.tile([6, 1], F32, space="PSUM")
        PE.matmul(out=pcb6[:], lhsT=onesf[0:1, 0:6], rhs=negcnt[:], start=True, stop=True)
        V.tensor_tensor(out=negvB6[:], in0=cf[0:6, 1:2], in1=pcb6[:], op=ALU.is_lt)

    # ---------------- selection indicator matmuls
    posidx_i = persist.tile([NPOS, 1], I32)
    negidxA_i = persist.tile([128, 1], I32)
    negidxB_i = persist.tile([6, 1], I32)
    with tc.tile_pool(name="indp", bufs=2) as xp, \
         tc.tile_pool(name="indpp", bufs=1, space="PSUM") as ppp:
        piP = ppp.tile([NPOS, 2], F32, space="PSUM", tag="piP")
        piNA = ppp.tile([128, 2], F32, space="PSUM", tag="piNA")
        piNB = ppp.tile([6, 2], F32, space="PSUM", tag="piNB")
        for t in range(NT):
            rg = xp.tile([PT, 1], F32, tag="rg")
            ind = xp.tile([PT, NNEG], F32, tag="ind")
            # positives
            V.tensor_tensor(out=rg[:], in0=rk[0:PT, 2 * t:2 * t + 1],
                            in1=offs[:, t:t + 1], op=ALU.add)
            V.tensor_scalar(out=ind[:, 0:NPOS], in0=kb[:, 0:NPOS], scalar1=rg[:],
                            scalar2=flagsPN[:, 2 * t:2 * t + 1], op0=ALU.is_equal, op1=ALU.mult)
            PE.matmul(out=piP[:], lhsT=ind[:, 0:NPOS], rhs=pidx2[0:PT, 2 * t:2 * t + 2],
                      start=(t == 0), stop=(t == NT - 1))
            # negatives
            V.tensor_tensor(out=rg[:], in0=rk[0:PT, 2 * t + 1:2 * t + 2],
                            in1=offs[:, NT + t:NT + t + 1], op=ALU.add)
            V.tensor_scalar(out=ind[:], in0=kb[:, 0:NNEG], scalar1=rg[:],
                            scalar2=flagsPN[:, 2 * t + 1:2 * t + 2], op0=ALU.is_equal, op1=ALU.mult)
            PE.matmul(out=piNA[:], lhsT=ind[:, 0:128], rhs=pidx2[0:PT, 2 * t:2 * t + 2],
                      start=(t == 0), stop=(t == NT - 1))
            PE.matmul(out=piNB[:], lhsT=ind[:, 128:NNEG], rhs=pidx2[0:PT, 2 * t:2 * t + 2],
                      start=(t == 0), stop=(t == NT - 1))
        tmpi = xp.tile([128, 1], F32, tag="tmpi")
        sps = xp.tile([128, 2], F32, tag="sps")
        V.tensor_copy(out=sps[0:NPOS, :], in_=piP[:])
        V.scalar_tensor_tensor(out=tmpi[0:NPOS, :], in0=sps[0:NPOS, 1:2], scalar=125.0,
                               in1=sps[0:NPOS, 0:1], op0=ALU.mult, op1=ALU.add)
        V.tensor_copy(out=posidx_i[:], in_=tmpi[0:NPOS, :])
        V.tensor_copy(out=sps[:], in_=piNA[:])
        V.scalar_tensor_tensor(out=tmpi[:], in0=sps[:, 1:2], scalar=125.0,
                               in1=sps[:, 0:1], op0=ALU.mult, op1=ALU.add)
        V.tensor_copy(out=negidxA_i[:], in_=tmpi[:])
        V.tensor_copy(out=sps[0:6, :], in_=piNB[:])
        V.scalar_tensor_tensor(out=tmpi[0:6, :], in0=sps[0:6, 1:2], scalar=125.0,
                               in1=sps[0:6, 0:1], op0=ALU.mult, op1=ALU.add)
        V.tensor_copy(out=negidxB_i[:], in_=tmpi[0:6, :])

    # ---------------- gathers
    posrois = persist.tile([NPOS, 4], F32)
    nc.gpsimd.indirect_dma_start(
        out=posrois[:], out_offset=None, in_=d["prop_d"][:],
        in_offset=bass.IndirectOffsetOnAxis(ap=posidx_i[:, 0:1], axis=0))
    V.tensor_scalar(out=posrois[:], in0=posrois[:], scalar1=posv[:], scalar2=None, op0=ALU.mult)

    negroisA = persist.tile([128, 4], F32)
    nc.gpsimd.indirect_dma_start(
        out=negroisA[:], out_offset=None, in_=d["prop_d"][:],
        in_offset=bass.IndirectOffsetOnAxis(ap=negidxA_i[:, 0:1], axis=0))
    V.tensor_scalar(out=negroisA[:], in0=negroisA[:], scalar1=negvA[:], scalar2=None, op0=ALU.mult)
    negroisB = persist.tile([6, 4], F32)
    nc.gpsimd.indirect_dma_start(
        out=negroisB[:], out_offset=None, in_=d["prop_d"][:],
        in_offset=bass.IndirectOffsetOnAxis(ap=negidxB_i[:, 0:1], axis=0))
    V.tensor_scalar(out=negroisB[:], in0=negroisB[:], scalar1=negvB6[:], scalar2=None, op0=ALU.mult)

    nc.sync.dma_start(d["orois_d"][0:NPOS, :], posrois[:])
    nc.sync.dma_start(d["orois_d"][NPOS:NPOS + 128, :], negroisA[:])
    nc.sync.dma_start(d["orois_d"][NPOS + 128:200, :], negroisB[:])

    # ---------------- roi_gt via fresh IoU on zeroed posrois
    roigt_i = persist.tile([NPOS, 1], I32)
    roigt_f = persist.tile([NPOS, 1], F32)
    with tc.tile_pool(name="ovg", bufs=1) as gp:
        iou = iou_rows(gp, NPOS, "g", posrois[:, 0:1], posrois[:, 1:2],
                       posrois[:, 2:3], posrois[:, 3:4])
        ovg = gp.tile([NPOS, 64], F32, tag="ovg")
        V.tensor_tensor(out=ovg[:], in0=iou[:], in1=BGTV[0:NPOS, :], op=ALU.mult)
        V.tensor_tensor(out=ovg[:], in0=ovg[:], in1=gtvm1[0:NPOS, :], op=ALU.add)
        rmax = gp.tile([NPOS, 1], F32, tag="grmax")
        V.tensor_reduce(rmax[:], ovg[:], AX.X, ALU.max)
        eq = gp.tile([NPOS, 64], F32, tag="geq")
        V.tensor_scalar(out=eq[:], in0=ovg[:], scalar1=rmax[:], scalar2=None, op0=ALU.is_equal)
        kb65 = gp.tile([NPOS, 64], F32, tag="gkb")
        V.tensor_scalar(out=kb65[:], in0=kb[0:NPOS, 0:64], scalar1=BIG, scalar2=None, op0=ALU.add)
        V.scalar_tensor_tensor(out=eq[:], in0=eq[:], scalar=-BIG, in1=kb65[:],
                               op0=ALU.mult, op1=ALU.add)
        V.tensor_reduce(roigt_f[:], eq[:], AX.X, ALU.min)
        V.tensor_copy(out=roigt_i[:], in_=roigt_f[:])

    gtb = persist.tile([NPOS, 4], F32)
    nc.gpsimd.indirect_dma_start(
        out=gtb[:], out_offset=None, in_=d["gt_d"][:],
        in_offset=bass.IndirectOffsetOnAxis(ap=roigt_i[:, 0:1], axis=0))
    clsg = persist.tile([NPOS, 1], F32)
    nc.gpsimd.indirect_dma_start(
        out=clsg[:], out_offset=None, in_=d["cls_d"][:],
        in_offset=bass.IndirectOffsetOnAxis(ap=roigt_i[:, 0:1], axis=0))
    V.tensor_tensor(out=clsg[:], in0=clsg[:], in1=posv[:], op=ALU.mult)
    nc.sync.dma_start(d["otcls_d"][0:NPOS, :], clsg[:])
    zfill = persist.tile([128, 8], F32)
    V.memset(zfill[:], 0.0)
    nc.sync.dma_start(d["otcls_d"][NPOS:194, :], zfill[:, 0:1])
    nc.sync.dma_start(d["otcls_d"][194:200, :], zfill[0:6, 1:2])
    nc.sync.dma_start(d["otdel_d"][NPOS:194, :], zfill[:, 2:6])
    nc.sync.dma_start(d["otdel_d"][194:200, :], zfill[0:6, 2:6])

    # ---------------- deltas
    with tc.tile_pool(name="del", bufs=1) as dp:
        def col(tile_, i):
            return tile_[:, i:i + 1]
        h = dp.tile([NPOS, 1], F32, tag="h")
        w = dp.tile([NPOS, 1], F32, tag="w")
        cy = dp.tile([NPOS, 1], F32, tag="cy")
        cx = dp.tile([NPOS, 1], F32, tag="cx")
        V.tensor_tensor(out=h[:], in0=col(posrois, 2), in1=col(posrois, 0), op=ALU.subtract)
        V.tensor_scalar(out=h[:], in0=h[:], scalar1=float(EPS), scalar2=None, op0=ALU.max)
        V.tensor_tensor(out=w[:], in0=col(posrois, 3), in1=col(posrois, 1), op=ALU.subtract)
        V.tensor_scalar(out=w[:], in0=w[:], scalar1=float(EPS), scalar2=None, op0=ALU.max)
        V.scalar_tensor_tensor(out=cy[:], in0=h[:], scalar=0.5, in1=col(posrois, 0),
                               op0=ALU.mult, op1=ALU.add)
        V.scalar_tensor_tensor(out=cx[:], in0=w[:], scalar=0.5, in1=col(posrois, 1),
                               op0=ALU.mult, op1=ALU.add)
        gh = dp.tile([NPOS, 1], F32, tag="gh")
        gw = dp.tile([NPOS, 1], F32, tag="gw")
        gcy = dp.tile([NPOS, 1], F32, tag="gcy")
        gcx = dp.tile([NPOS, 1], F32, tag="gcx")
        V.tensor_tensor(out=gh[:], in0=col(gtb, 2), in1=col(gtb, 0), op=ALU.subtract)
        V.tensor_scalar(out=gh[:], in0=gh[:], scalar1=float(EPS), scalar2=None, op0=ALU.max)
        V.tensor_tensor(out=gw[:], in0=col(gtb, 3), in1=col(gtb, 1), op=ALU.subtract)
        V.tensor_scalar(out=gw[:], in0=gw[:], scalar1=float(EPS), scalar2=None, op0=ALU.max)
        V.scalar_tensor_tensor(out=gcy[:], in0=gh[:], scalar=0.5, in1=col(gtb, 0),
                               op0=ALU.mult, op1=ALU.add)
        V.scalar_tensor_tensor(out=gcx[:], in0=gw[:], scalar=0.5, in1=col(gtb, 1),
                               op0=ALU.mult, op1=ALU.add)
        dl = dp.tile([NPOS, 4], F32, tag="dl")
        rh = dp.tile([NPOS, 1], F32, tag="rh")
        rw = dp.tile([NPOS, 1], F32, tag="rw")
        V.reciprocal(out=rh[:], in_=h[:])
        V.reciprocal(out=rw[:], in_=w[:])
        V.tensor_tensor(out=col(dl, 0), in0=gcy[:], in1=cy[:], op=ALU.subtract)
        V.tensor_tensor(out=col(dl, 0), in0=col(dl, 0), in1=rh[:], op=ALU.mult)
        V.tensor_tensor(out=col(dl, 1), in0=gcx[:], in1=cx[:], op=ALU.subtract)
        V.tensor_tensor(out=col(dl, 1), in0=col(dl, 1), in1=rw[:], op=ALU.mult)
        V.tensor_tensor(out=col(dl, 2), in0=gh[:], in1=rh[:], op=ALU.mult)
        V.tensor_tensor(out=col(dl, 3), in0=gw[:], in1=rw[:], op=ALU.mult)
        S.activation(out=dl[:, 2:4], in_=dl[:, 2:4], func=ACT.Ln)
        V.tensor_scalar(out=dl[:, 0:2], in0=dl[:, 0:2], scalar1=10.0, scalar2=None, op0=ALU.mult)
        V.tensor_scalar(out=dl[:, 2:4], in0=dl[:, 2:4], scalar1=5.0, scalar2=None, op0=ALU.mult)
        V.tensor_scalar(out=dl[:], in0=dl[:], scalar1=posv[:], scalar2=None, op0=ALU.mult)
        nc.sync.dma_start(d["otdel_d"][0:NPOS, :], dl[:])

    # ---------------- mask sampling coords
    crd = persist.tile([NPOS, 8 * MH], F32)  # y0f y1f x0l x1l ly omly lx omlx
    Y0F, Y1F = crd[:, 0:28], crd[:, 28:56]
    X0L, X1L = crd[:, 56:84], crd[:, 84:112]
    LY, OMLY = crd[:, 112:140], crd[:, 140:168]
    LX, OMLX = crd[:, 168:196], crd[:, 196:224]
    xw0 = persist.tile([NPOS, 1], F32)

    with tc.tile_pool(name="crd", bufs=1) as cp:
        ys = cp.tile([NPOS, MH], F32, tag="ys")
        dd = cp.tile([NPOS, 1], F32, tag="dd")
        # ys
        V.tensor_tensor(out=dd[:], in0=posrois[:, 2:3], in1=posrois[:, 0:1], op=ALU.subtract)
        V.tensor_scalar(out=ys[:], in0=iyrep[:], scalar1=dd[:], scalar2=posrois[:, 0:1],
                        op0=ALU.mult, op1=ALU.add)
        V.tensor_scalar(out=ys[:], in0=ys[:], scalar1=511.0, scalar2=None, op0=ALU.mult)
        V.tensor_scalar(out=Y0F, in0=ys[:], scalar1=-0.5, scalar2=12582912.0,
                        op0=ALU.add, op1=ALU.add)
        V.tensor_scalar(out=Y0F, in0=Y0F, scalar1=-12582912.0, scalar2=0.0,
                        op0=ALU.add, op1=ALU.add)  # +0.0 canonicalizes -0.0
        V.tensor_tensor(out=LY, in0=ys[:], in1=Y0F, op=ALU.subtract)
        V.tensor_scalar(out=Y1F, in0=Y0F, scalar1=1.0, scalar2=511.0, op0=ALU.add, op1=ALU.min)
        V.tensor_scalar(out=OMLY, in0=LY, scalar1=-1.0, scalar2=1.0, op0=ALU.mult, op1=ALU.add)
        # xs
        V.tensor_tensor(out=dd[:], in0=posrois[:, 3:4], in1=posrois[:, 1:2], op=ALU.subtract)
        V.tensor_scalar(out=ys[:], in0=iyrep[:], scalar1=dd[:], scalar2=posrois[:, 1:2],
                        op0=ALU.mult, op1=ALU.add)
        V.tensor_scalar(out=ys[:], in0=ys[:], scalar1=511.0, scalar2=None, op0=ALU.mult)
        V.tensor_scalar(out=X0L, in0=ys[:], scalar1=-0.5, scalar2=12582912.0,
                        op0=ALU.add, op1=ALU.add)
        V.tensor_scalar(out=X0L, in0=X0L, scalar1=-12582912.0, scalar2=0.0,
                        op0=ALU.add, op1=ALU.add)
        V.tensor_tensor(out=LX, in0=ys[:], in1=X0L, op=ALU.subtract)  # X0L = x0f global for now
        V.tensor_scalar(out=X1L, in0=X0L, scalar1=1.0, scalar2=511.0, op0=ALU.add, op1=ALU.min)
        V.tensor_scalar(out=OMLX, in0=LX, scalar1=-1.0, scalar2=1.0, op0=ALU.mult, op1=ALU.add)
        # window
        V.tensor_tensor(out=xw0[:], in0=crd[:, 56:57], in1=crd[:, 83:84], op=ALU.min)
        V.tensor_scalar(out=xw0[:], in0=xw0[:], scalar1=0.0, scalar2=float(512 - WIN),
                        op0=ALU.max, op1=ALU.min)
        # localize x coords
        V.tensor_scalar(out=X0L, in0=X0L, scalar1=xw0[:], scalar2=0.0,
                        op0=ALU.subtract, op1=ALU.add)  # +0.0 kills -0.0 for is_equal
        V.tensor_scalar(out=X1L, in0=X1L, scalar1=xw0[:], scalar2=0.0,
                        op0=ALU.subtract, op1=ALU.add)

    # ---------------- permutation of per-slot data
    bunA = persist.tile([NPOS, 112], F32)   # y0f|y1f|x0l|x1l rows permuted
    bunrow = persist.tile([1, NPOS * 112], F32)  # flattened for matmul rhs
    scT = persist.tile([MH, 4 * NPOS], F32)  # lyT|omlyT|lxT|omlxT columns per slot
    bi32x = persist.tile([1, NPOS], I32)     # xw0 per slot (free dim)
    bi32g = persist.tile([1, NPOS], I32)     # gt index per slot
    pvb = persist.tile([MH, NPOS], F32)
    with tc.tile_pool(name="prm", bufs=1, space="PSUM") as qp, \
         tc.tile_pool(name="prms", bufs=1) as qs:
        pa = qp.tile([112, NPOS], F32, space="PSUM", tag="pa")
        PE.transpose(out=pa[:], in_=crd[:, 0:112], identity=perm[:])
        pasb = qs.tile([112, NPOS], F32)
        V.tensor_copy(out=pasb[:], in_=pa[:])
        pa2 = qp.tile([NPOS, 112], F32, space="PSUM", tag="pa2")
        PE.transpose(out=pa2[:], in_=pasb[:], identity=idf[0:112, 0:112])
        V.tensor_copy(out=bunA[:], in_=pa2[:])
        nc.sync.dma_start(bunrow[:], bunA[:])

        for i4, sl in enumerate((LY, OMLY, LX, OMLX)):
            pt_ = qp.tile([MH, NPOS], F32, space="PSUM", tag="pt")
            PE.transpose(out=pt_[:], in_=sl, identity=perm[:])
            V.tensor_copy(out=scT[:, i4 * NPOS:(i4 + 1) * NPOS], in_=pt_[:])

        pbx = qp.tile([1, NPOS], F32, space="PSUM", tag="pbx")
        PE.transpose(out=pbx[:], in_=xw0[:], identity=perm[:])
        V.tensor_copy(out=bi32x[:], in_=pbx[:])
        pbg = qp.tile([1, NPOS], F32, space="PSUM", tag="pbg")
        PE.transpose(out=pbg[:], in_=roigt_f[:], identity=perm[:])
        V.tensor_copy(out=bi32g[:], in_=pbg[:])

        pvrow = qp.tile([1, NPOS], F32, space="PSUM", tag="pvrow")
        PE.transpose(out=pvrow[:], in_=posv[:], identity=perm[:])
        pvr_sb = qs.tile([1, NPOS], F32)
        V.tensor_copy(out=pvr_sb[:], in_=pvrow[:])
        ppvb = qp.tile([MH, NPOS], F32, space="PSUM", tag="ppvb")
        PE.matmul(out=ppvb[:], lhsT=onesf[0:1, 0:MH], rhs=pvr_sb[:], start=True, stop=True)
        V.tensor_copy(out=pvb[:], in_=ppvb[:])

    LYT = scT[:, 0:NPOS]
    OMLYT = scT[:, NPOS:2 * NPOS]
    LXT = scT[:, 2 * NPOS:3 * NPOS]
    OMLXT = scT[:, 3 * NPOS:4 * NPOS]

    # ---------------- per-slot mask generation
    outm = persist.tile([MH, 33 * MH], F32)
    with tc.tile_pool(name="msk", bufs=2) as mp, \
         tc.tile_pool(name="mskp", bufs=1, space="PSUM") as mq, \
         tc.tile_pool(name="mskp1", bufs=1, space="PSUM") as mq1:
        for k in range(33):
            xw0v = vload(bi32x[0:1, k:k + 1], 0, 512 - WIN)
            gv = vload(bi32g[0:1, k:k + 1], 0, 63)

            pybc = mq1.tile([128, 56], F32, space="PSUM", tag="pbc2", bufs=2)
            PE.matmul(out=pybc[:], lhsT=onesf[0:1, :],
                      rhs=bunrow[0:1, 112 * k:112 * k + 56], start=True, stop=True)
            pxbc = mq1.tile([128, 56], F32, space="PSUM", tag="pbc2", bufs=2)
            PE.matmul(out=pxbc[:], lhsT=onesf[0:1, :],
                      rhs=bunrow[0:1, 112 * k + 56:112 * k + 112], start=True, stop=True)

            sx1 = mp.tile([128, 64], BF16, tag="sx1")
            V.memset(sx1[:, 28:32], 0.0)
            V.memset(sx1[:, 60:64], 0.0)
            V.tensor_scalar(out=sx1[:, 0:28], in0=pxbc[:, 0:28], scalar1=cf[:, 0:1],
                            scalar2=None, op0=ALU.is_equal)
            V.tensor_scalar(out=sx1[:, 32:60], in0=pxbc[:, 28:56], scalar1=cf[:, 0:1],
                            scalar2=None, op0=ALU.is_equal)
            sx2 = mp.tile([64, 64], BF16, tag="sx2")
            V.memset(sx2[:, 28:32], 0.0)
            V.memset(sx2[:, 60:64], 0.0)
            V.tensor_scalar(out=sx2[:, 0:28], in0=pxbc[0:64, 0:28], scalar1=cf[0:64, 1:2],
                            scalar2=None, op0=ALU.is_equal)
            V.tensor_scalar(out=sx2[:, 32:60], in0=pxbc[0:64, 28:56], scalar1=cf[0:64, 1:2],
                            scalar2=None, op0=ALU.is_equal)

            pG = mq.tile([56, WIN], F32, space="PSUM", tag="pG", bufs=2)
            for b in range(4):
                pl = mp.tile([128, WIN], BF16, tag="pl")
                V.tensor_copy(out=pl[:],
                              in_=mask_sb[b][:, ds(xw0v, WIN), ds(gv, 1)].rearrange(
                                  "p a b -> p (a b)"))
                sy = mp.tile([128, 56], BF16, tag="sy")
                V.tensor_scalar(out=sy[:], in0=pybc[:], scalar1=cf[:, b:b + 1],
                                scalar2=None, op0=ALU.is_equal)
                PE.matmul(out=pG[:], lhsT=sy[:], rhs=pl[:], start=(b == 0), stop=(b == 3))

            gsb = mp.tile([56, WIN], BF16, tag="gsb")
            V.tensor_copy(out=gsb[:], in_=pG[:])
            pgt1 = mq.tile([128, 56], BF16, space="PSUM", tag="pgt", bufs=2)
            PE.transpose(out=pgt1[:], in_=gsb[:, 0:128], identity=idh[0:56, 0:56])
            pgt2_t = mq.tile([128, 56], BF16, space="PSUM", tag="pgt", bufs=2, name="pgt2")
            pgt2 = pgt2_t[0:64, :]
            PE.transpose(out=pgt2[:], in_=gsb[:, 128:WIN], identity=idh[0:56, 0:56])
            gt1 = mp.tile([128, 56], BF16, tag="gt1")
            V.tensor_copy(out=gt1[:], in_=pgt1[:])
            gt2 = mp.tile([64, 56], BF16, tag="gt2")
            V.tensor_copy(out=gt2[:], in_=pgt2[:])

            pH = mq.tile([64, 56], F32, space="PSUM", tag="pH", bufs=1)
            PE.matmul(out=pH[:], lhsT=sx1[:], rhs=gt1[:], start=True, stop=False)
            PE.matmul(out=pH[:], lhsT=sx2[:], rhs=gt2[:], start=False, stop=True)

            # H[j-part(x0@0|x1@32), i-free(G0|G1)]: m00=H[0:28,0:28] m10=H[0:28,28:56]
            #                                       m01=H[32:60,0:28] m11=H[32:60,28:56]
            t1 = mp.tile([MH, MH], F32, tag="t1")
            top = mp.tile([MH, MH], F32, tag="top")
            bot = mp.tile([MH, MH], F32, tag="bot")
            V.tensor_scalar(out=t1[:], in0=pH[32:60, 0:28], scalar1=LXT[:, k:k + 1],
                            scalar2=None, op0=ALU.mult)
            V.scalar_tensor_tensor(out=top[:], in0=pH[0:28, 0:28], scalar=OMLXT[:, k:k + 1],
                                   in1=t1[:], op0=ALU.mult, op1=ALU.add)
            V.tensor_scalar(out=t1[:], in0=pH[32:60, 28:56], scalar1=LXT[:, k:k + 1],
                            scalar2=None, op0=ALU.mult)
            V.scalar_tensor_tensor(out=bot[:], in0=pH[0:28, 28:56], scalar=OMLXT[:, k:k + 1],
                                   in1=t1[:], op0=ALU.mult, op1=ALU.add)

            ptt = mq.tile([MH, 2 * MH], F32, space="PSUM", tag="ptt", bufs=1)
            PE.transpose(out=ptt[:, 0:MH], in_=top[:], identity=idf[0:MH, 0:MH])
            PE.transpose(out=ptt[:, MH:2 * MH], in_=bot[:], identity=idf[0:MH, 0:MH])
            V.tensor_scalar(out=t1[:], in0=ptt[:, MH:2 * MH], scalar1=LYT[:, k:k + 1],
                            scalar2=None, op0=ALU.mult)
            vv = mp.tile([MH, MH], F32, tag="vv")
            V.scalar_tensor_tensor(out=vv[:], in0=ptt[:, 0:MH], scalar=OMLYT[:, k:k + 1],
                                   in1=t1[:], op0=ALU.mult, op1=ALU.add)
            V.tensor_scalar(out=vv[:], in0=vv[:], scalar1=0.5, scalar2=None, op0=ALU.is_gt)
            V.tensor_scalar(out=outm[:, k * MH:(k + 1) * MH], in0=vv[:],
                            scalar1=pvb[:, k:k + 1], scalar2=None, op0=ALU.mult)

    nc.sync.dma_start(
        d["omask_d"][:].rearrange("k i j -> i k j"),
        outm[:].rearrange("p (k j) -> p k j", k=33))

    stack.close()


# ---------------------------------------------------------------- host entry
_NC_CACHE = None


def _get_nc():
    global _NC_CACHE
    if _NC_CACHE is None:
        nc = build_nc()
        if not nc.is_finalized():
            nc.finalize()  # Bacc: runs alloc_regs; axon exec path skips it
        # The neuron NEFF cache keys on the HLO module hash, which does not
        # cover the bass program payload — key the cache dir by program hash
        # so program edits can't hit a stale NEFF.
        import hashlib
        import os
        h = hashlib.sha256(nc.to_pretty_json_str().encode()).hexdigest()[:16]
        os.environ["NEURON_COMPILE_CACHE_URL"] = f"/tmp/neuron_cache_{h}"
        _NC_CACHE = nc
    return _NC_CACHE


def make_in_maps(proposals, prior_class_ids, prior_boxes, prior_masks):
    consts = {p: host_constants(p) for p in (0, 1)}
    in_maps = []
    for c in range(8):
        i, par = c // 2, c % 2
        cs = consts[par]
        in_maps.append({
            "prop": np.ascontiguousarray(proposals[i], np.float32),
            "gt": np.ascontiguousarray(prior_boxes[i], np.float32),
            "clsf": np.ascontiguousarray(prior_class_ids[i], np.float32).reshape(NG, 1),
            "masks": np.ascontiguousarray(np.asarray(prior_masks[i]).view(np.uint8)),
            "cf": cs["cf"], "trio": cs["trio"], "onesf": cs["onesf"],
            "iotarow": cs["iotarow"], "pidx2": cs["pidx2"], "negtab": cs["negtab"],
            "perm": cs["perm"], "idf": cs["idf"],
            "idh": cs["idh"],  # f32 host array; uploaded as bf16 param
            "iyrep": cs["iyrep"],
        })
    return in_maps


def assemble(results):
    import ml_dtypes  # noqa: F401
    rois = np.stack([results[2 * i]["orois"] for i in range(4)])
    tcls = np.stack([results[2 * i]["otcls"][:, 0] for i in range(4)])
    tdel = np.stack([results[2 * i]["otdel"] for i in range(4)])
    masks = np.zeros((4, 200, MH, MH), np.float32)
    for i in range(4):
        for par in (0, 1):
            om = results[2 * i + par]["omask"]
            masks[i, par:NPOS:2] = om
    return (rois.astype(np.float32), np.rint(tcls).astype(np.int32),
            tdel.astype(np.float32), masks)


def _run(proposals, prior_class_ids, prior_boxes, prior_masks, **kw):
    nc = _get_nc()
    in_maps = make_in_maps(proposals, prior_class_ids, prior_boxes, prior_masks)
    import ml_dtypes
    for m in in_maps:
        m["idh"] = m["idh"].astype(ml_dtypes.bfloat16)
    return run_bass_kernel_spmd(nc, in_maps, list(range(8)), **kw)


def kernel(proposals, prior_class_ids, prior_boxes, prior_masks):
    res = _run(proposals, prior_class_ids, prior_boxes, prior_masks)
    return assemble(res.results)


def kernel_traced(proposals, prior_class_ids, prior_boxes, prior_masks):
    res = _run(proposals, prior_class_ids, prior_boxes, prior_masks,
               trace=True, trace_cores=list(range(8)))
    ns = res.exec_time_ns
    return assemble(res.results), ns, res
